# revision 1
# baseline (speedup 1.0000x reference)
"""Causal single-head attention (N=4096, D=F=1024) on 8 TRN2 NeuronCores.

Sequence-parallel sharding: core c owns query rows [512c, 512(c+1)).

Two SPMD launches:
  A) QKV projection — each core computes q/k/v for its own 512 rows
     (weights replicated, host pre-transposed to contraction-major layouts).
  B) attention + output projection — each core consumes its q.T plus
     full k.T / v that the host has right-aligned ("rotated") so that the
     causal diagonal always lands in the LAST 512-column block regardless
     of core id. Junk key columns are zeroed (their exp(0)=1 probabilities
     multiply zeroed v rows and a zeroed ones-column, so they contribute
     nothing); the remaining triangular mask is core-invariant and applied
     on-chip with affine_select. This keeps the SPMD program fully uniform
     across cores with no control flow.

Matmul operands are bf16 (full PE rate, f32 PSUM accumulation); k/v/weight
tensors are host-pre-blocked so every DMA is ~128 descriptors of >=2KB
contiguous per partition.
"""

import sys

try:
    import concourse.bass as bass
except ImportError:  # pragma: no cover
    sys.path.insert(0, "/opt/trn_rl_repo")
    import concourse.bass as bass

import ml_dtypes
import numpy as np

import concourse.mybir as mybir
import concourse.tile as tile
from concourse import bacc
from concourse.bass_utils import run_bass_kernel_spmd

N, D, F = 4096, 1024, 1024
C = 8              # cores
NL = N // C        # 512 query rows per core
P = 128
SCALE = 1.0 / float(np.sqrt(np.float32(F)))

F32 = mybir.dt.float32
MM_DT = mybir.dt.bfloat16  # matmul operand dtype (PSUM accumulation stays f32)

DT = D // P        # 8 contraction tiles
FT = F // P        # 8 f tiles
MT = N // P        # 32 key tiles
NT2 = NL // P      # 4 query-row tiles per core

# Filled with [launchA_ns, launchB_ns] when BASS_TRACE=1 profiling is active.
LAST_EXEC_NS = [None, None]
LAST_RESULTS = [None, None]

_CACHE = {}


def _mm(x):
    return x


def _build_qkv():
    nc = bacc.Bacc(None, target_bir_lowering=False)
    xT = nc.dram_tensor("xT", [P, DT, NL], MM_DT, kind="ExternalInput")
    wqb = nc.dram_tensor("wqb", [FT, P, DT, P], MM_DT, kind="ExternalInput")
    wkb = nc.dram_tensor("wkb", [FT, P, DT, P], MM_DT, kind="ExternalInput")
    wvb = nc.dram_tensor("wvb", [2, P, DT, 512], MM_DT, kind="ExternalInput")
    bq = nc.dram_tensor("bq", [P, FT], F32, kind="ExternalInput")
    bk = nc.dram_tensor("bk", [P, FT], F32, kind="ExternalInput")
    bvB = nc.dram_tensor("bvB", [P, F], F32, kind="ExternalInput")
    qT_o = nc.dram_tensor("qT_o", [F, NL], MM_DT, kind="ExternalOutput")
    kT_o = nc.dram_tensor("kT_o", [F, NL], MM_DT, kind="ExternalOutput")
    v_o = nc.dram_tensor("v_o", [NL, F], MM_DT, kind="ExternalOutput")

    with tile.TileContext(nc) as tc:
        with (
            tc.tile_pool(name="singles", bufs=1) as singles,
            tc.tile_pool(name="weights", bufs=8) as weights,
            tc.tile_pool(name="osb", bufs=6) as opool,
            tc.tile_pool(name="psum", bufs=6, space="PSUM") as psum,
        ):
            warm = singles.tile([P, NL], MM_DT)
            nc.vector.memset(warm, 0.0)
            wps = psum.tile([P, NL], F32, tag="ps")
            for wi in range(24):
                nc.tensor.matmul(
                    wps,
                    warm[:, :P],
                    warm,
                    start=(wi == 0),
                    stop=(wi == 23),
                )
            xT_sb = singles.tile([P, DT, NL], MM_DT)
            nc.sync.dma_start(out=xT_sb[:, : DT // 2, :], in_=xT.ap()[:, : DT // 2, :])
            nc.scalar.dma_start(
                out=xT_sb[:, DT // 2 :, :], in_=xT.ap()[:, DT // 2 :, :]
            )
            bq_sb = singles.tile([P, FT], F32)
            nc.sync.dma_start(out=bq_sb, in_=bq.ap())
            bk_sb = singles.tile([P, FT], F32)
            nc.sync.dma_start(out=bk_sb, in_=bk.ap())
            bvB_sb = singles.tile([P, F], F32)
            nc.sync.dma_start(out=bvB_sb, in_=bvB.ap())

            # q.T / k.T : out[f_tile, n] = sum_d wT[d, f] * xT[d, n]
            # weights streamed in per-f-tile chunks so PE starts early
            for w_t, b_sb, out_t in ((wqb, bq_sb, qT_o), (wkb, bk_sb, kT_o)):
                for ft in range(FT):
                    wc = weights.tile([P, DT, P], MM_DT, tag="wc")
                    nc.sync.dma_start(out=wc, in_=w_t.ap()[ft])
                    ps = psum.tile([P, NL], F32, tag="ps")
                    for dt_i in range(DT):
                        nc.tensor.matmul(
                            ps,
                            _mm(wc[:, dt_i, :]),
                            _mm(xT_sb[:, dt_i, :]),
                            start=(dt_i == 0),
                            stop=(dt_i == DT - 1),
                        )
                    osb = opool.tile([P, NL], MM_DT, tag="osb")
                    nc.vector.tensor_scalar_add(
                        out=osb, in0=ps, scalar1=b_sb[:, ft : ft + 1]
                    )
                    nc.scalar.dma_start(
                        out=out_t.ap()[ft * P : (ft + 1) * P, :], in_=osb
                    )

            # v : out[m_tile, f] = sum_d xT[d, m] * wvT[d, f]
            for fc in range(2):
                fs = slice(fc * 512, (fc + 1) * 512)
                wvc = weights.tile([P, DT, 512], MM_DT, tag="wvc")
                nc.sync.dma_start(out=wvc, in_=wvb.ap()[fc])
                for mi in range(NT2):
                    ps = psum.tile([P, 512], F32, tag="ps")
                    for dt_i in range(DT):
                        nc.tensor.matmul(
                            ps,
                            _mm(xT_sb[:, dt_i, mi * P : (mi + 1) * P]),
                            _mm(wvc[:, dt_i, :]),
                            start=(dt_i == 0),
                            stop=(dt_i == DT - 1),
                        )
                    vsb = opool.tile([P, 512], MM_DT, tag="osb")
                    nc.vector.tensor_add(out=vsb, in0=ps, in1=bvB_sb[:, fs])
                    nc.scalar.dma_start(
                        out=v_o.ap()[mi * P : (mi + 1) * P, fs], in_=vsb
                    )
    nc.finalize()
    return nc


def _build_attn():
    nc = bacc.Bacc(None, target_bir_lowering=False)
    qT = nc.dram_tensor("qT", [P, FT, NL], MM_DT, kind="ExternalInput")
    kbs = nc.dram_tensor("kbs", [MT, P, FT, P], MM_DT, kind="ExternalInput")
    vbk = nc.dram_tensor("vbk", [FT, 2, P, MT // 2, P], MM_DT, kind="ExternalInput")
    ones = nc.dram_tensor("ones", [P, MT], MM_DT, kind="ExternalInput")
    projT = nc.dram_tensor("projT", [F, F], MM_DT, kind="ExternalInput")
    pbB = nc.dram_tensor("pbB", [P, F], F32, kind="ExternalInput")
    out_o = nc.dram_tensor("out_o", [NL, F], F32, kind="ExternalOutput")

    with tile.TileContext(nc) as tc:
        with (
            tc.tile_pool(name="singles", bufs=1) as singles,
            tc.tile_pool(name="kc", bufs=10) as kpool,
            tc.tile_pool(name="pt", bufs=MT) as ptpool,
            tc.tile_pool(name="vc", bufs=6) as vpool,
            tc.tile_pool(name="osb", bufs=3) as opool,
            tc.tile_pool(name="sps", bufs=3, space="PSUM") as spsum,
            tc.tile_pool(name="rps", bufs=1, space="PSUM") as rpsum,
            tc.tile_pool(name="zps", bufs=2, space="PSUM") as zpsum,
            tc.tile_pool(name="ops", bufs=2, space="PSUM") as opsum,
            tc.tile_pool(name="dram", bufs=1, space="DRAM") as drampool,
        ):
            warm = singles.tile([P, NL], MM_DT)
            nc.vector.memset(warm, 0.0)
            wps = spsum.tile([P, NL], F32, tag="sps")
            for wi in range(24):
                nc.tensor.matmul(
                    wps,
                    warm[:, :P],
                    warm,
                    start=(wi == 0),
                    stop=(wi == 23),
                )
            qT_sb = singles.tile([P, FT, NL], MM_DT)
            nc.scalar.dma_start(
                out=qT_sb[:, : FT // 2, :], in_=qT.ap()[:, : FT // 2, :]
            )
            nc.sync.dma_start(
                out=qT_sb[:, FT // 2 :, :], in_=qT.ap()[:, FT // 2 :, :]
            )
            ones_sb = singles.tile([P, MT], MM_DT)
            nc.scalar.dma_start(out=ones_sb, in_=ones.ap())

            # ---- scores + exp:  pT[m, n] = exp(SCALE * sum_f kTr[f, m] qT[f, n])
            pts = []
            chunks = [("s", i, 1) for i in range(MT)]
            mt0 = 0
            for kind, idx, csz in chunks:
                kc = kpool.tile([P, FT, P], MM_DT, tag="kc")
                if idx >= 3 and idx % 2 == 1:
                    nc.scalar.dma_start(out=kc, in_=kbs.ap()[idx])
                else:
                    nc.sync.dma_start(out=kc, in_=kbs.ap()[idx])
                for mi in range(csz):
                    mt = mt0 + mi
                    ps = spsum.tile([P, NL], F32, tag="sps")
                    for ft in range(FT):
                        nc.tensor.matmul(
                            ps,
                            _mm(kc[:, ft, :]),
                            _mm(qT_sb[:, ft, :]),
                            start=(ft == 0),
                            stop=(ft == FT - 1),
                        )
                    pt = ptpool.tile([P, NL], MM_DT, tag="pt")
                    nc.scalar.activation(
                        out=pt,
                        in_=ps,
                        func=mybir.ActivationFunctionType.Exp,
                        scale=SCALE,
                    )
                    if mt >= MT - NT2:
                        # diagonal block: keep only m_local <= n
                        nc.gpsimd.affine_select(
                            out=pt,
                            in_=pt,
                            pattern=[[1, NL]],
                            compare_op=mybir.AluOpType.is_ge,
                            fill=0.0,
                            base=-(mt - (MT - NT2)) * P,
                            channel_multiplier=-1,
                        )
                    pts.append(pt)
                mt0 += csz

            # ---- row sums (over valid keys only) via ones-column matmuls
            rps = rpsum.tile([1, NL], F32)
            for mt in range(MT):
                nc.tensor.matmul(
                    rps,
                    _mm(ones_sb[:, mt : mt + 1]),
                    _mm(pts[mt]),
                    start=(mt == 0),
                    stop=(mt == MT - 1),
                )
            recip_row = singles.tile([1, NL], F32)
            nc.vector.reciprocal(out=recip_row, in_=rps)
            scratch = drampool.tile([1, NL], F32)
            nc.sync.dma_start(out=scratch, in_=recip_row)
            recip_np = singles.tile([P, NT2], F32)
            nc.sync.dma_start(
                out=recip_np, in_=scratch[0].rearrange("(t p) -> p t", p=P)
            )

            # ---- z.T[f, n] = sum_m v[m, f] * pT[m, n]
            pbB_sb = singles.tile([P, F], F32)
            projT_sb = singles.tile([P, FT, F], MM_DT)
            z_tiles = []
            for ft in range(FT):
                if ft == 0:
                    nc.gpsimd.dma_start(out=pbB_sb, in_=pbB.ap())
                if ft == 1:
                    nc.gpsimd.dma_start(
                        out=projT_sb,
                        in_=projT.ap().rearrange("(t p) f -> p t f", p=P),
                    )
                for vh in range(2):  # half-chunks of 16 key tiles
                    vc = vpool.tile([P, MT // 2, P], MM_DT, tag="vc")
                    if ft < 4:
                        nc.gpsimd.dma_start(out=vc, in_=vbk.ap()[ft, vh])
                    else:
                        nc.sync.dma_start(out=vc, in_=vbk.ap()[ft, vh])
                    if vh == 0:
                        zps = zpsum.tile([P, NL], F32, tag="zps")
                    for mi in range(MT // 2):
                        mt = vh * 16 + mi
                        nc.tensor.matmul(
                            zps,
                            _mm(vc[:, mi, :]),
                            _mm(pts[mt]),
                            start=(mt == 0),
                            stop=(mt == MT - 1),
                        )
                zt = singles.tile([P, NL], MM_DT, tag=f"z{ft}")
                nc.vector.tensor_copy(out=zt, in_=zps)
                z_tiles.append(zt)

            # ---- out[n, o] = (z.T/rowsum) @ projT + pb
            for nt in range(NT2):
                for oc in range(2):
                    os_ = slice(oc * 512, (oc + 1) * 512)
                    ops = opsum.tile([P, 512], F32, tag="ops")
                    for ft in range(FT):
                        nc.tensor.matmul(
                            ops,
                            _mm(z_tiles[ft][:, nt * P : (nt + 1) * P]),
                            _mm(projT_sb[:, ft, os_]),
                            start=(ft == 0),
                            stop=(ft == FT - 1),
                        )
                    osb = opool.tile([P, 512], F32, tag="osb")
                    nc.vector.scalar_tensor_tensor(
                        out=osb,
                        in0=ops,
                        scalar=recip_np[:, nt : nt + 1],
                        in1=pbB_sb[:, os_],
                        op0=mybir.AluOpType.mult,
                        op1=mybir.AluOpType.add,
                    )
                    nc.sync.dma_start(
                        out=out_o.ap()[nt * P : (nt + 1) * P, os_], in_=osb
                    )
    nc.finalize()
    return nc


def _get_programs():
    if "qkv" not in _CACHE:
        _CACHE["qkv"] = _build_qkv()
        _CACHE["attn"] = _build_attn()
    return _CACHE["qkv"], _CACHE["attn"]


def _c(a):
    return np.ascontiguousarray(a, dtype=np.float32)


def _b(a):
    return np.ascontiguousarray(np.asarray(a, dtype=np.float32).astype(ml_dtypes.bfloat16))


def kernel(x, wq_w, wq_b, wk_w, wk_b, wv_w, wv_b, proj_w, proj_b):
    x = np.asarray(x, dtype=np.float32)
    nc_qkv, nc_attn = _get_programs()

    # ---- launch A: QKV projection, sequence-sharded
    xT = np.asarray(x, dtype=np.float32).T        # [D, N]
    # blocked weight layouts: [FT, P, DT, P] so every chunk DMA is linear
    wqb = _b(np.asarray(wq_w).T.reshape(DT, P, FT, P).transpose(2, 1, 0, 3))
    wkb = _b(np.asarray(wk_w).T.reshape(DT, P, FT, P).transpose(2, 1, 0, 3))
    wvb = _b(np.asarray(wv_w).T.reshape(DT, P, 2, 512).transpose(2, 1, 0, 3))
    bq_pb = _c(np.asarray(wq_b).reshape(FT, P).T)   # [P, FT]
    bk_pb = _c(np.asarray(wk_b).reshape(FT, P).T)
    bvB = _c(np.broadcast_to(np.asarray(wv_b), (P, F)))
    in_a = []
    for c in range(C):
        xT_blk = _b(
            xT[:, c * NL : (c + 1) * NL].reshape(DT, P, NL).transpose(1, 0, 2)
        )
        in_a.append(
            {
                "xT": xT_blk,
                "wqb": wqb,
                "wkb": wkb,
                "wvb": wvb,
                "bq": bq_pb,
                "bk": bk_pb,
                "bvB": bvB,
            }
        )
    res_a = run_bass_kernel_spmd(nc_qkv, in_a, core_ids=list(range(C)))
    LAST_EXEC_NS[0] = res_a.exec_time_ns
    LAST_RESULTS[0] = res_a

    kT_full = np.concatenate([res_a.results[c]["kT_o"] for c in range(C)], axis=1)
    v_full = np.concatenate([res_a.results[c]["v_o"] for c in range(C)], axis=0)

    # ---- launch B: attention + projection
    projT = _b(np.asarray(proj_w).T)              # [F, F]
    pbB = _c(np.broadcast_to(np.asarray(proj_b), (P, F)))
    in_b = []
    for c in range(C):
        L = NL * (c + 1)          # valid key rows for this core
        J = N - L                 # zero-padded junk columns (multiple of 512)
        qT_blk = np.ascontiguousarray(res_a.results[c]["qT_o"].reshape(FT, P, NL).transpose(1, 0, 2))
        kTr = np.zeros((F, N), dtype=ml_dtypes.bfloat16)
        kTr[:, J:] = kT_full[:, :L]
        ka = kTr.reshape(FT, P, MT, P)
        kbs = np.ascontiguousarray(ka.transpose(2, 1, 0, 3))
        vr = np.zeros((N, F), dtype=ml_dtypes.bfloat16)
        vr[J:, :] = v_full[:L]
        vbk = np.ascontiguousarray(
            vr.reshape(2, MT // 2, P, FT, P).transpose(3, 0, 2, 1, 4)
        )
        ones_pb = np.zeros((P, MT), dtype=ml_dtypes.bfloat16)
        ones_pb[:, J // P :] = 1.0
        in_b.append(
            {
                "qT": qT_blk,
                "kbs": kbs,
                "vbk": vbk,
                "ones": ones_pb,
                "projT": projT,
                "pbB": pbB,
            }
        )
    res_b = run_bass_kernel_spmd(nc_attn, in_b, core_ids=list(range(C)))
    LAST_EXEC_NS[1] = res_b.exec_time_ns
    LAST_RESULTS[1] = res_b

    return np.concatenate([res_b.results[c]["out_o"] for c in range(C)], axis=0)



# revision 3
# speedup vs baseline: 1.0259x; 1.0259x over previous
"""Causal single-head attention (N=4096, D=F=1024) on 8 TRN2 NeuronCores.

Sequence-parallel with causal load balancing: query tiles (128 rows) are
assigned round-robin — core c owns tiles {c, 8+c, 16+c, 24+c}, one per
"slot" k=0..3.  Slot k only attends key tiles [0, 8*(k+1)), so attention
matmul work drops to 80/128 of the dense-causal-ignoring version while
the SPMD program stays uniform across cores (the per-core diagonal
position is handled by per-core mask DATA, not control flow).

Two SPMD launches:
  A) QKV projection — each core computes q/k/v for its own 4 query tiles
     (weights replicated; host pre-transposes to contraction-major).
  B) attention + output projection — chunk-major: for key tile m the
     scores matmul covers all still-eligible slots at once, so the free
     dim is 512/384/256/128 (wide free keeps the PE's LDWEIGHTS hidden).
     k/v live SBUF-resident and are shared across slots (nested key
     ranges).  att@v runs as two ft-half passes to fit zT in 4 PSUM
     banks.  Row sums come from a ones-column matmul; the reciprocal is
     transposed to query-partition form via a DRAM round trip hidden
     under the output-projection matmuls.

Matmul operands are bf16 (f32 PSUM accumulation); all big DMAs are
host-pre-blocked so each is ~128 descriptors of >=2KB contiguous.
"""

import sys

try:
    import concourse.bass as bass
except ImportError:  # pragma: no cover
    sys.path.insert(0, "/opt/trn_rl_repo")
    import concourse.bass as bass

import ml_dtypes
import numpy as np

import concourse.mybir as mybir
import concourse.tile as tile
from concourse import bacc
from concourse.bass_utils import run_bass_kernel_spmd

N, D, F = 4096, 1024, 1024
C = 8              # cores
NL = N // C        # 512 query rows per core
P = 128
SCALE = 1.0 / float(np.sqrt(np.float32(F)))

F32 = mybir.dt.float32
MM_DT = mybir.dt.bfloat16  # matmul operand dtype (PSUM accumulation stays f32)

DT = D // P        # 8 contraction tiles
FT = F // P        # 8 f tiles
MT = N // P        # 32 key tiles
SLOTS = NL // P    # 4 query tiles (slots) per core
CI = MT // 8       # 4 key chunks of 8 key tiles

# Filled with [launchA_ns, launchB_ns] when BASS_TRACE=1 profiling is active.
LAST_EXEC_NS = [None, None]
LAST_RESULTS = [None, None]

_CACHE = {}


def _build_qkv():
    nc = bacc.Bacc(None, target_bir_lowering=False)
    xT = nc.dram_tensor("xT", [P, DT, NL], MM_DT, kind="ExternalInput")
    wqb = nc.dram_tensor("wqb", [FT, P, DT, P], MM_DT, kind="ExternalInput")
    wkb = nc.dram_tensor("wkb", [FT, P, DT, P], MM_DT, kind="ExternalInput")
    wvb = nc.dram_tensor("wvb", [2, P, DT, 512], MM_DT, kind="ExternalInput")
    bq = nc.dram_tensor("bq", [P, FT], F32, kind="ExternalInput")
    bk = nc.dram_tensor("bk", [P, FT], F32, kind="ExternalInput")
    bvB = nc.dram_tensor("bvB", [P, F], F32, kind="ExternalInput")
    qT_o = nc.dram_tensor("qT_o", [F, NL], MM_DT, kind="ExternalOutput")
    kT_o = nc.dram_tensor("kT_o", [F, NL], MM_DT, kind="ExternalOutput")
    v_o = nc.dram_tensor("v_o", [NL, F], MM_DT, kind="ExternalOutput")

    with tile.TileContext(nc) as tc:
        with (
            tc.tile_pool(name="singles", bufs=1) as singles,
            tc.tile_pool(name="weights", bufs=8) as weights,
            tc.tile_pool(name="osb", bufs=6) as opool,
            tc.tile_pool(name="psum", bufs=6, space="PSUM") as psum,
        ):
            warm = singles.tile([P, NL], MM_DT)
            nc.vector.memset(warm, 0.0)
            wps = psum.tile([P, NL], F32, tag="ps")
            for wi in range(24):
                nc.tensor.matmul(
                    wps,
                    warm[:, :P],
                    warm,
                    start=(wi == 0),
                    stop=(wi == 23),
                )
            xT_sb = singles.tile([P, DT, NL], MM_DT)
            nc.sync.dma_start(out=xT_sb[:, : DT // 2, :], in_=xT.ap()[:, : DT // 2, :])
            nc.scalar.dma_start(
                out=xT_sb[:, DT // 2 :, :], in_=xT.ap()[:, DT // 2 :, :]
            )
            bq_sb = singles.tile([P, FT], F32)
            nc.sync.dma_start(out=bq_sb, in_=bq.ap())
            bk_sb = singles.tile([P, FT], F32)
            nc.sync.dma_start(out=bk_sb, in_=bk.ap())
            bvB_sb = singles.tile([P, F], F32)
            nc.sync.dma_start(out=bvB_sb, in_=bvB.ap())

            # q.T / k.T : out[f_tile, n] = sum_d wT[d, f] * xT[d, n]
            # weights streamed in per-f-tile chunks so PE starts early
            for w_t, b_sb, out_t in ((wqb, bq_sb, qT_o), (wkb, bk_sb, kT_o)):
                for ft in range(FT):
                    wc = weights.tile([P, DT, P], MM_DT, tag="wc")
                    nc.sync.dma_start(out=wc, in_=w_t.ap()[ft])
                    ps = psum.tile([P, NL], F32, tag="ps")
                    for dt_i in range(DT):
                        nc.tensor.matmul(
                            ps,
                            wc[:, dt_i, :],
                            xT_sb[:, dt_i, :],
                            start=(dt_i == 0),
                            stop=(dt_i == DT - 1),
                        )
                    osb = opool.tile([P, NL], MM_DT, tag="osb")
                    nc.vector.tensor_scalar_add(
                        out=osb, in0=ps, scalar1=b_sb[:, ft : ft + 1]
                    )
                    nc.scalar.dma_start(
                        out=out_t.ap()[ft * P : (ft + 1) * P, :], in_=osb
                    )

            # v : out[m_tile, f] = sum_d xT[d, m] * wvT[d, f]
            for fc in range(2):
                fs = slice(fc * 512, (fc + 1) * 512)
                wvc = weights.tile([P, DT, 512], MM_DT, tag="wvc")
                nc.sync.dma_start(out=wvc, in_=wvb.ap()[fc])
                for mi in range(SLOTS):
                    ps = psum.tile([P, 512], F32, tag="ps")
                    for dt_i in range(DT):
                        nc.tensor.matmul(
                            ps,
                            xT_sb[:, dt_i, mi * P : (mi + 1) * P],
                            wvc[:, dt_i, :],
                            start=(dt_i == 0),
                            stop=(dt_i == DT - 1),
                        )
                    vsb = opool.tile([P, 512], MM_DT, tag="osb")
                    nc.vector.tensor_add(out=vsb, in0=ps, in1=bvB_sb[:, fs])
                    nc.scalar.dma_start(
                        out=v_o.ap()[mi * P : (mi + 1) * P, fs], in_=vsb
                    )
    nc.finalize()
    return nc


def _build_attn():
    nc = bacc.Bacc(None, target_bir_lowering=False)
    qT = nc.dram_tensor("qT", [P, FT, NL], MM_DT, kind="ExternalInput")
    # kb[ci, p, ft, j] = k[ci*1024 + j, ft*128 + p]
    kb = nc.dram_tensor("kb", [CI, P, FT, 1024], MM_DT, kind="ExternalInput")
    # vb[ci, p, u, f] = v[(8*ci+u)*128 + p, f]
    vb = nc.dram_tensor("vb", [CI, P, 8, F], MM_DT, kind="ExternalInput")
    # maskb[p, u, q]: per-core diagonal-region masks (ones / tril / zeros)
    maskb = nc.dram_tensor("maskb", [P, 8, P], MM_DT, kind="ExternalInput")
    # projTb[p, t, f] = proj_w.T[t*128+p, f]
    projTb = nc.dram_tensor("projTb", [P, FT, F], MM_DT, kind="ExternalInput")
    pbB = nc.dram_tensor("pbB", [P, F], F32, kind="ExternalInput")
    out_o = nc.dram_tensor("out_o", [NL, F], F32, kind="ExternalOutput")

    with tile.TileContext(nc) as tc:
        with (
            tc.tile_pool(name="singles", bufs=1) as singles,
            tc.tile_pool(name="osb", bufs=3) as opool,
            tc.tile_pool(name="sps", bufs=3, space="PSUM") as spsum,
            tc.tile_pool(name="zps", bufs=4, space="PSUM") as zpsum,
            tc.tile_pool(name="rps", bufs=1, space="PSUM") as rpsum,
            tc.tile_pool(name="dram", bufs=1, space="DRAM") as drampool,
        ):
            warm = singles.tile([P, 512], MM_DT)
            nc.vector.memset(warm, 0.0)
            wps = zpsum.tile([P, 512], F32, tag="zps")
            for wi in range(24):
                nc.tensor.matmul(
                    wps,
                    warm[:, :P],
                    warm,
                    start=(wi == 0),
                    stop=(wi == 23),
                )

            # ---- resident inputs
            qT_sb = singles.tile([P, FT, NL], MM_DT)
            nc.scalar.dma_start(out=qT_sb, in_=qT.ap())
            k_sb = []
            for ci in range(CI):
                kc = singles.tile([P, FT, 1024], MM_DT, name=f"k_sb{ci}")
                nc.sync.dma_start(out=kc, in_=kb.ap()[ci])
                k_sb.append(kc)
            v_sb = []
            for ci in range(CI):
                vc = singles.tile([P, 8, F], MM_DT, name=f"v_sb{ci}")
                nc.gpsimd.dma_start(out=vc, in_=vb.ap()[ci])
                v_sb.append(vc)
            masks_sb = singles.tile([P, 8, P], MM_DT)
            nc.scalar.dma_start(out=masks_sb, in_=maskb.ap())
            projT_sb = singles.tile([P, FT, F], MM_DT)
            nc.scalar.dma_start(out=projT_sb, in_=projTb.ap())
            pbB_sb = singles.tile([P, F], F32)
            nc.scalar.dma_start(out=pbB_sb, in_=pbB.ap())
            ones_sb = singles.tile([P, 1], MM_DT)
            nc.vector.memset(ones_sb, 1.0)

            # pt arenas (bf16 attention weights), one per key chunk
            pt_ar = [
                singles.tile([P, 8, (CI - ci) * P], MM_DT, name=f"pt{ci}")
                for ci in range(CI)
            ]
            # zT arena: z^T[f, q] bf16, [P, ft, 512]
            zT_sb = singles.tile([P, FT, NL], MM_DT)
            rps = rpsum.tile([1, NL], F32)
            zA = [zpsum.tile([P, NL], F32, tag="zps", name=f"zA{h}") for h in range(4)]

            def scores(m):
                ci, u = divmod(m, 8)
                W = (CI - ci) * P
                ps = spsum.tile([P, W], F32, tag="sps")
                for ft in range(FT):
                    nc.tensor.matmul(
                        ps,
                        k_sb[ci][:, ft, u * P : (u + 1) * P],
                        qT_sb[:, ft, ci * P : NL],
                        start=(ft == 0),
                        stop=(ft == FT - 1),
                    )
                pt = pt_ar[ci][:, u, :]
                nc.scalar.activation(
                    out=pt,
                    in_=ps,
                    func=mybir.ActivationFunctionType.Exp,
                    scale=SCALE,
                )
                # mask only the first 128 columns (slot ci — its diagonal chunk)
                nc.vector.tensor_mul(
                    out=pt_ar[ci][:, u, :P],
                    in0=pt_ar[ci][:, u, :P],
                    in1=masks_sb[:, u, :],
                )

            def attv(m, zt, fts):
                ci, u = divmod(m, 8)
                pt = pt_ar[ci][:, u, :]
                for i, ft in enumerate(fts):
                    nc.tensor.matmul(
                        zt[i][:, ci * P : NL],
                        v_sb[ci][:, u, ft * P : (ft + 1) * P],
                        pt,
                        start=(m == 0),
                        stop=(m == MT - 1),
                        skip_group_check=True,
                    )

            def rowsum(m):
                ci, u = divmod(m, 8)
                nc.tensor.matmul(
                    rps[:, ci * P : NL],
                    ones_sb,
                    pt_ar[ci][:, u, :],
                    start=(m == 0),
                    stop=(m == MT - 1),
                    skip_group_check=True,
                )

            # ---- pass A: scores + exp + mask + att@v (ft 0..3) + rowsums
            scores(0)
            for m in range(1, MT):
                scores(m)
                attv(m - 1, zA, range(4))
                rowsum(m - 1)
            attv(MT - 1, zA, range(4))
            rowsum(MT - 1)

            # reciprocal of row sums; transpose to [q-part, slot] via DRAM
            recip_row = singles.tile([1, NL], F32)
            nc.vector.reciprocal(out=recip_row, in_=rps)
            scratch = drampool.tile([1, NL], F32)
            nc.scalar.dma_start(out=scratch, in_=recip_row)
            recip_np = singles.tile([P, SLOTS], F32)
            nc.scalar.dma_start(
                out=recip_np, in_=scratch[0].rearrange("(t p) -> p t", p=P)
            )

            # drain zT (ft 0..3)
            for h in range(4):
                if h % 2 == 0:
                    nc.vector.tensor_copy(out=zT_sb[:, h, :], in_=zA[h])
                else:
                    nc.scalar.activation(
                        out=zT_sb[:, h, :],
                        in_=zA[h],
                        func=mybir.ActivationFunctionType.Copy,
                    )

            # ---- pass B: att@v (ft 4..7)
            zB = [zpsum.tile([P, NL], F32, tag="zps", name=f"zB{h}") for h in range(4)]
            for m in range(MT):
                attv(m, zB, range(4, FT))
            for h in range(4):
                if h % 2 == 0:
                    nc.vector.tensor_copy(out=zT_sb[:, 4 + h, :], in_=zB[h])
                else:
                    nc.scalar.activation(
                        out=zT_sb[:, 4 + h, :],
                        in_=zB[h],
                        func=mybir.ActivationFunctionType.Copy,
                    )

            # ---- out[n, o] = (zT/rowsum) @ projT + pb
            for kk in range(SLOTS):
                for oc in range(2):
                    os_ = slice(oc * 512, (oc + 1) * 512)
                    ops = zpsum.tile([P, 512], F32, tag="zps", name="ops")
                    for ft in range(FT):
                        nc.tensor.matmul(
                            ops,
                            zT_sb[:, ft, kk * P : (kk + 1) * P],
                            projT_sb[:, ft, os_],
                            start=(ft == 0),
                            stop=(ft == FT - 1),
                        )
                    osb = opool.tile([P, 512], F32, tag="osb")
                    nc.vector.scalar_tensor_tensor(
                        out=osb,
                        in0=ops,
                        scalar=recip_np[:, kk : kk + 1],
                        in1=pbB_sb[:, os_],
                        op0=mybir.AluOpType.mult,
                        op1=mybir.AluOpType.add,
                    )
                    nc.sync.dma_start(
                        out=out_o.ap()[kk * P : (kk + 1) * P, os_], in_=osb
                    )
    nc.finalize()
    return nc


def _get_programs():
    if "qkv" not in _CACHE:
        _CACHE["qkv"] = _build_qkv()
        _CACHE["attn"] = _build_attn()
    return _CACHE["qkv"], _CACHE["attn"]


def _c(a):
    return np.ascontiguousarray(a, dtype=np.float32)


def _b(a):
    return np.ascontiguousarray(np.asarray(a, dtype=np.float32).astype(ml_dtypes.bfloat16))


def kernel(x, wq_w, wq_b, wk_w, wk_b, wv_w, wv_b, proj_w, proj_b):
    x = np.asarray(x, dtype=np.float32)
    nc_qkv, nc_attn = _get_programs()

    # ---- launch A: QKV projection; core c owns query tiles {c, 8+c, 16+c, 24+c}
    # blocked weight layouts: [FT, P, DT, P] so every chunk DMA is linear
    wqb = _b(np.asarray(wq_w).T.reshape(DT, P, FT, P).transpose(2, 1, 0, 3))
    wkb = _b(np.asarray(wk_w).T.reshape(DT, P, FT, P).transpose(2, 1, 0, 3))
    wvb = _b(np.asarray(wv_w).T.reshape(DT, P, 2, 512).transpose(2, 1, 0, 3))
    bq_pb = _c(np.asarray(wq_b).reshape(FT, P).T)   # [P, FT]
    bk_pb = _c(np.asarray(wk_b).reshape(FT, P).T)
    bvB = _c(np.broadcast_to(np.asarray(wv_b), (P, F)))
    x_t = x.reshape(MT, P, D)                       # [tile, row, d]
    in_a = []
    for c in range(C):
        rows = x_t[c::C].reshape(NL, D)             # tiles c, 8+c, 16+c, 24+c
        xT_blk = _b(rows.T.reshape(DT, P, NL).transpose(1, 0, 2))
        in_a.append(
            {
                "xT": xT_blk,
                "wqb": wqb,
                "wkb": wkb,
                "wvb": wvb,
                "bq": bq_pb,
                "bk": bk_pb,
                "bvB": bvB,
            }
        )
    res_a = run_bass_kernel_spmd(nc_qkv, in_a, core_ids=list(range(C)))
    LAST_EXEC_NS[0] = res_a.exec_time_ns
    LAST_RESULTS[0] = res_a

    # reassemble full k/v in natural row order (tile index = 8*s + c)
    kT_all = np.stack(
        [np.asarray(res_a.results[c]["kT_o"]).reshape(F, SLOTS, P) for c in range(C)]
    )  # [c, F, s, P]
    kT_full = kT_all.transpose(1, 2, 0, 3).reshape(F, N)
    v_all = np.stack(
        [np.asarray(res_a.results[c]["v_o"]).reshape(SLOTS, P, F) for c in range(C)]
    )  # [c, s, P, F]
    v_full = v_all.transpose(1, 0, 2, 3).reshape(N, F)

    # ---- launch B: attention + projection (chunk-blocked, replicated k/v)
    kb = np.ascontiguousarray(
        kT_full.reshape(FT, P, CI, 1024).transpose(2, 1, 0, 3)
    )
    vb = np.ascontiguousarray(v_full.reshape(CI, 8, P, F).transpose(0, 2, 1, 3))
    projTb = _b(np.asarray(proj_w).T.reshape(FT, P, F).transpose(1, 0, 2))
    pbB = _c(np.broadcast_to(np.asarray(proj_b), (P, F)))
    tri = (np.arange(P)[:, None] <= np.arange(P)[None, :])  # key p <= query q
    in_b = []
    for c in range(C):
        qT_blk = np.ascontiguousarray(
            np.asarray(res_a.results[c]["qT_o"]).reshape(FT, P, NL).transpose(1, 0, 2)
        )
        maskb = np.zeros((P, 8, P), dtype=ml_dtypes.bfloat16)
        maskb[:, :c, :] = 1.0
        maskb[:, c, :] = tri.astype(ml_dtypes.bfloat16)
        in_b.append(
            {
                "qT": qT_blk,
                "kb": kb,
                "vb": vb,
                "maskb": maskb,
                "projTb": projTb,
                "pbB": pbB,
            }
        )
    res_b = run_bass_kernel_spmd(nc_attn, in_b, core_ids=list(range(C)))
    LAST_EXEC_NS[1] = res_b.exec_time_ns
    LAST_RESULTS[1] = res_b

    out_all = np.stack(
        [np.asarray(res_b.results[c]["out_o"]).reshape(SLOTS, P, F) for c in range(C)]
    )  # [c, k, P, F]
    return out_all.transpose(1, 0, 2, 3).reshape(N, F)


# revision 12
# speedup vs baseline: 1.1200x; 1.0917x over previous
"""Causal single-head attention (N=4096, D=F=1024) on 8 TRN2 NeuronCores.

Sequence-parallel with causal load balancing: query tiles (128 rows) are
assigned round-robin — core c owns tiles {c, 8+c, 16+c, 24+c}, one per
"slot" k=0..3.  Slot k only attends key tiles [0, 8*(k+1)), so attention
matmul work drops to 80/128 of the dense-causal-ignoring version while
the SPMD program stays uniform across cores (the per-core diagonal
position is handled by per-core mask DATA, not control flow).

Two SPMD launches:
  A) QKV projection — each core computes q/k/v for its own 4 query tiles
     (weights replicated; host pre-transposes to contraction-major).
  B) attention + output projection — chunk-major: for key tile m the
     scores matmul covers all still-eligible slots at once, so the free
     dim is 512/384/256/128 (wide free keeps the PE's LDWEIGHTS hidden).
     k/v live SBUF-resident and are shared across slots (nested key
     ranges).  att@v runs as two ft-half passes to fit zT in 4 PSUM
     banks.  Row sums come from a ones-column matmul; the reciprocal is
     transposed to query-partition form via a DRAM round trip hidden
     under the output-projection matmuls.

Matmul operands are bf16 (f32 PSUM accumulation); all big DMAs are
host-pre-blocked so each is ~128 descriptors of >=2KB contiguous.
"""

import sys

try:
    import concourse.bass as bass
except ImportError:  # pragma: no cover
    sys.path.insert(0, "/opt/trn_rl_repo")
    import concourse.bass as bass

import ml_dtypes
import numpy as np

import concourse.mybir as mybir
import concourse.tile as tile
from concourse import bacc
from concourse.bass_utils import run_bass_kernel_spmd

N, D, F = 4096, 1024, 1024
C = 8              # cores
NL = N // C        # 512 query rows per core
P = 128
SCALE = 1.0 / float(np.sqrt(np.float32(F)))

F32 = mybir.dt.float32
MM_DT = mybir.dt.bfloat16  # matmul operand dtype (PSUM accumulation stays f32)

DT = D // P        # 8 contraction tiles
FT = F // P        # 8 f tiles
MT = N // P        # 32 key tiles
SLOTS = NL // P    # 4 query tiles (slots) per core
CI = MT // 8       # 4 key chunks of 8 key tiles

# Filled with [launchA_ns, launchB_ns] when BASS_TRACE=1 profiling is active.
LAST_EXEC_NS = [None, None]
LAST_RESULTS = [None, None]

_CACHE = {}


def _build_qkv():
    nc = bacc.Bacc(None, target_bir_lowering=False)
    xT = nc.dram_tensor("xT", [P, DT, NL], MM_DT, kind="ExternalInput")
    wqb = nc.dram_tensor("wqb", [FT, P, DT, P], MM_DT, kind="ExternalInput")
    wkb = nc.dram_tensor("wkb", [FT, P, DT, P], MM_DT, kind="ExternalInput")
    wvb = nc.dram_tensor("wvb", [2, P, DT, 512], MM_DT, kind="ExternalInput")
    bq = nc.dram_tensor("bq", [P, FT], F32, kind="ExternalInput")
    bk = nc.dram_tensor("bk", [P, FT], F32, kind="ExternalInput")
    bvB = nc.dram_tensor("bvB", [P, F], F32, kind="ExternalInput")
    qT_o = nc.dram_tensor("qT_o", [F, NL], MM_DT, kind="ExternalOutput")
    kT_o = nc.dram_tensor("kT_o", [F, NL], MM_DT, kind="ExternalOutput")
    v_o = nc.dram_tensor("v_o", [NL, F], MM_DT, kind="ExternalOutput")

    with tile.TileContext(nc) as tc:
        with (
            tc.tile_pool(name="singles", bufs=1) as singles,
            tc.tile_pool(name="weights", bufs=8) as weights,
            tc.tile_pool(name="osb", bufs=6) as opool,
            tc.tile_pool(name="psum", bufs=6, space="PSUM") as psum,
        ):
            warm = singles.tile([P, NL], MM_DT)
            nc.vector.memset(warm, 0.0)
            wps = psum.tile([P, NL], F32, tag="ps")
            for wi in range(24):
                nc.tensor.matmul(
                    wps,
                    warm[:, :P],
                    warm,
                    start=(wi == 0),
                    stop=(wi == 23),
                )
            xT_sb = singles.tile([P, DT, NL], MM_DT)
            nc.sync.dma_start(out=xT_sb[:, : DT // 2, :], in_=xT.ap()[:, : DT // 2, :])
            nc.scalar.dma_start(
                out=xT_sb[:, DT // 2 :, :], in_=xT.ap()[:, DT // 2 :, :]
            )
            bq_sb = singles.tile([P, FT], F32)
            nc.sync.dma_start(out=bq_sb, in_=bq.ap())
            bk_sb = singles.tile([P, FT], F32)
            nc.sync.dma_start(out=bk_sb, in_=bk.ap())
            bvB_sb = singles.tile([P, F], F32)
            nc.sync.dma_start(out=bvB_sb, in_=bvB.ap())

            # q.T / k.T : out[f_tile, n] = sum_d wT[d, f] * xT[d, n]
            # weights streamed in per-f-tile chunks so PE starts early
            for w_t, b_sb, out_t in ((wqb, bq_sb, qT_o), (wkb, bk_sb, kT_o)):
                for ft in range(FT):
                    wc = weights.tile([P, DT, P], MM_DT, tag="wc")
                    nc.sync.dma_start(out=wc, in_=w_t.ap()[ft])
                    ps = psum.tile([P, NL], F32, tag="ps")
                    for dt_i in range(DT):
                        nc.tensor.matmul(
                            ps,
                            wc[:, dt_i, :],
                            xT_sb[:, dt_i, :],
                            start=(dt_i == 0),
                            stop=(dt_i == DT - 1),
                        )
                    osb = opool.tile([P, NL], MM_DT, tag="osb")
                    nc.vector.tensor_scalar_add(
                        out=osb, in0=ps, scalar1=b_sb[:, ft : ft + 1]
                    )
                    nc.scalar.dma_start(
                        out=out_t.ap()[ft * P : (ft + 1) * P, :], in_=osb
                    )

            # v : out[m_tile, f] = sum_d xT[d, m] * wvT[d, f]
            for fc in range(2):
                fs = slice(fc * 512, (fc + 1) * 512)
                wvc = weights.tile([P, DT, 512], MM_DT, tag="wvc")
                nc.sync.dma_start(out=wvc, in_=wvb.ap()[fc])
                for mi in range(SLOTS):
                    ps = psum.tile([P, 512], F32, tag="ps")
                    for dt_i in range(DT):
                        nc.tensor.matmul(
                            ps,
                            xT_sb[:, dt_i, mi * P : (mi + 1) * P],
                            wvc[:, dt_i, :],
                            start=(dt_i == 0),
                            stop=(dt_i == DT - 1),
                        )
                    vsb = opool.tile([P, 512], MM_DT, tag="osb")
                    nc.vector.tensor_add(out=vsb, in0=ps, in1=bvB_sb[:, fs])
                    nc.scalar.dma_start(
                        out=v_o.ap()[mi * P : (mi + 1) * P, fs], in_=vsb
                    )
    nc.finalize()
    return nc


def _build_attn():
    nc = bacc.Bacc(None, target_bir_lowering=False)
    qT = nc.dram_tensor("qT", [P, FT, NL], MM_DT, kind="ExternalInput")
    # kb[ci, p, u, ft, j] = k[(8*ci+u)*128 + j, ft*128 + p]
    kb = nc.dram_tensor("kb", [CI, P, 8, FT, P], MM_DT, kind="ExternalInput")
    # vb[ci, p, u, f] = v[(8*ci+u)*128 + p, f]
    vb = nc.dram_tensor("vb", [CI, P, 8, F], MM_DT, kind="ExternalInput")
    # maskb[p, u, q]: per-core diagonal-region masks (ones / tril / zeros)
    maskb = nc.dram_tensor("maskb", [P, 8, P], MM_DT, kind="ExternalInput")
    # projTb[p, t, f] = proj_w.T[t*128+p, f]
    projTb = nc.dram_tensor("projTb", [P, FT, F], MM_DT, kind="ExternalInput")
    pbB = nc.dram_tensor("pbB", [P, F], F32, kind="ExternalInput")
    out_o = nc.dram_tensor("out_o", [NL, F], F32, kind="ExternalOutput")

    with tile.TileContext(nc) as tc:
        with (
            tc.tile_pool(name="singles", bufs=1) as singles,
            tc.tile_pool(name="osb", bufs=3) as opool,
            tc.tile_pool(name="sps", bufs=3, space="PSUM") as spsum,
            tc.tile_pool(name="zps", bufs=4, space="PSUM") as zpsum,
            tc.tile_pool(name="rps", bufs=1, space="PSUM") as rpsum,
            tc.tile_pool(name="dram", bufs=1, space="DRAM") as drampool,
        ):
            warm = singles.tile([P, 512], MM_DT)
            nc.vector.memset(warm, 0.0)
            wps = zpsum.tile([P, 512], F32, tag="zps")
            for wi in range(12):
                nc.tensor.matmul(
                    wps,
                    warm[:, :P],
                    warm,
                    start=(wi == 0),
                    stop=(wi == 11),
                )

            # ---- resident inputs.  The critical first-window tensors (masks,
            # q, k chunk 0, v chunk 0) are sub-chunked so the first score
            # matmuls start ~4 us in and stream behind the DMAs.
            masks_sb = singles.tile([P, 8, P], MM_DT)
            nc.scalar.dma_start(out=masks_sb, in_=maskb.ap())
            qT_sb = singles.tile([P, FT, NL], MM_DT)
            nc.scalar.dma_start(out=qT_sb, in_=qT.ap())
            # 1 MB sub-chunks: big enough to amortize per-DMA overhead,
            # small enough that chunk ci's head arrives just-in-time
            k_sb = []
            for ci in range(CI):
                kc = singles.tile([P, 8, FT, P], MM_DT, name=f"k_sb{ci}")
                nc.sync.dma_start(out=kc[:, :4], in_=kb.ap()[ci, :, :4])
                nc.sync.dma_start(out=kc[:, 4:], in_=kb.ap()[ci, :, 4:])
                k_sb.append(kc)
            v_sb = []
            for ci in range(CI):
                vc = singles.tile([P, 8, F], MM_DT, name=f"v_sb{ci}")
                nc.gpsimd.dma_start(out=vc[:, :4], in_=vb.ap()[ci, :, :4])
                nc.gpsimd.dma_start(out=vc[:, 4:], in_=vb.ap()[ci, :, 4:])
                v_sb.append(vc)
            # gpsimd's DMA queue is serial, so these naturally wait behind
            # the v transfers — out of the critical first-window bandwidth
            projT_sb = singles.tile([P, FT, F], MM_DT)
            nc.gpsimd.dma_start(out=projT_sb, in_=projTb.ap())
            pbB_sb = singles.tile([P, F], F32)
            nc.gpsimd.dma_start(out=pbB_sb, in_=pbB.ap())
            ones_sb = singles.tile([P, 1], MM_DT)
            nc.vector.memset(ones_sb, 1.0)

            # pt arenas (bf16 attention weights), one per key chunk
            pt_ar = [
                singles.tile([P, 8, (CI - ci) * P], MM_DT, name=f"pt{ci}")
                for ci in range(CI)
            ]
            # zT arena: z^T[f, q] bf16, [P, ft, 512]
            zT_sb = singles.tile([P, FT, NL], MM_DT)
            rps = rpsum.tile([1, NL], F32)
            zA = [zpsum.tile([P, NL], F32, tag="zps", name=f"zA{h}") for h in range(4)]

            def scores(m):
                ci, u = divmod(m, 8)
                W = (CI - ci) * P
                ps = spsum.tile([P, W], F32, tag="sps")
                for ft in range(FT):
                    nc.tensor.matmul(
                        ps,
                        k_sb[ci][:, u, ft, :],
                        qT_sb[:, ft, ci * P : NL],
                        start=(ft == 0),
                        stop=(ft == FT - 1),
                    )
                pt = pt_ar[ci][:, u, :]
                nc.scalar.activation(
                    out=pt,
                    in_=ps,
                    func=mybir.ActivationFunctionType.Exp,
                    scale=SCALE,
                )
                # mask only the first 128 columns (slot ci — its diagonal chunk)
                nc.vector.tensor_mul(
                    out=pt_ar[ci][:, u, :P],
                    in0=pt_ar[ci][:, u, :P],
                    in1=masks_sb[:, u, :],
                )

            def attv(m, zt, fts):
                ci, u = divmod(m, 8)
                pt = pt_ar[ci][:, u, :]
                for i, ft in enumerate(fts):
                    nc.tensor.matmul(
                        zt[i][:, ci * P : NL],
                        v_sb[ci][:, u, ft * P : (ft + 1) * P],
                        pt,
                        start=(m == 0),
                        stop=(m == MT - 1),
                        skip_group_check=True,
                    )

            def rowsum(m):
                ci, u = divmod(m, 8)
                nc.tensor.matmul(
                    rps[:, ci * P : NL],
                    ones_sb,
                    pt_ar[ci][:, u, :],
                    start=(m == 0),
                    stop=(m == MT - 1),
                    skip_group_check=True,
                )

            # ---- pass A: scores + exp + mask + att@v (ft 0..3) + rowsums
            scores(0)
            for m in range(1, MT):
                scores(m)
                attv(m - 1, zA, range(4))
                rowsum(m - 1)
            attv(MT - 1, zA, range(4))
            rowsum(MT - 1)

            # row sums -> [q-part, slot] via DRAM round trip, then reciprocal
            rsum_row = singles.tile([1, NL], F32)
            nc.vector.tensor_copy(out=rsum_row, in_=rps)
            scratch = drampool.tile([1, NL], F32)
            nc.scalar.dma_start(out=scratch, in_=rsum_row)
            rsum_np = singles.tile([P, SLOTS], F32)
            nc.scalar.dma_start(
                out=rsum_np, in_=scratch[0].rearrange("(t p) -> p t", p=P)
            )
            recip_np = singles.tile([P, SLOTS], F32)
            nc.vector.reciprocal(out=recip_np, in_=rsum_np)

            # drain zT (ft 0..3)
            for h in range(4):
                if h % 2 == 0:
                    nc.vector.tensor_copy(out=zT_sb[:, h, :], in_=zA[h])
                else:
                    nc.scalar.activation(
                        out=zT_sb[:, h, :],
                        in_=zA[h],
                        func=mybir.ActivationFunctionType.Copy,
                    )

            # ---- pass B: att@v (ft 4..7)
            zB = [zpsum.tile([P, NL], F32, tag="zps", name=f"zB{h}") for h in range(4)]
            for m in range(MT):
                attv(m, zB, range(4, FT))
            for h in range(4):
                if h % 2 == 0:
                    nc.vector.tensor_copy(out=zT_sb[:, 4 + h, :], in_=zB[h])
                else:
                    nc.scalar.activation(
                        out=zT_sb[:, 4 + h, :],
                        in_=zB[h],
                        func=mybir.ActivationFunctionType.Copy,
                    )

            # ---- out[n, o] = (zT/rowsum) @ projT + pb
            for kk in range(SLOTS):
                for oc in range(2):
                    os_ = slice(oc * 512, (oc + 1) * 512)
                    ops = zpsum.tile([P, 512], F32, tag="zps", name="ops")
                    for ft in range(FT):
                        nc.tensor.matmul(
                            ops,
                            zT_sb[:, ft, kk * P : (kk + 1) * P],
                            projT_sb[:, ft, os_],
                            start=(ft == 0),
                            stop=(ft == FT - 1),
                        )
                    osb = opool.tile([P, 512], F32, tag="osb")
                    nc.vector.scalar_tensor_tensor(
                        out=osb,
                        in0=ops,
                        scalar=recip_np[:, kk : kk + 1],
                        in1=pbB_sb[:, os_],
                        op0=mybir.AluOpType.mult,
                        op1=mybir.AluOpType.add,
                    )
                    nc.sync.dma_start(
                        out=out_o.ap()[kk * P : (kk + 1) * P, os_], in_=osb
                    )
    nc.finalize()
    return nc


def _get_programs():
    if "qkv" not in _CACHE:
        _CACHE["qkv"] = _build_qkv()
        _CACHE["attn"] = _build_attn()
    return _CACHE["qkv"], _CACHE["attn"]


def _c(a):
    return np.ascontiguousarray(a, dtype=np.float32)


def _b(a):
    return np.ascontiguousarray(np.asarray(a, dtype=np.float32).astype(ml_dtypes.bfloat16))


def kernel(x, wq_w, wq_b, wk_w, wk_b, wv_w, wv_b, proj_w, proj_b):
    x = np.asarray(x, dtype=np.float32)
    nc_qkv, nc_attn = _get_programs()

    # ---- launch A: QKV projection; core c owns query tiles {c, 8+c, 16+c, 24+c}
    # blocked weight layouts: [FT, P, DT, P] so every chunk DMA is linear
    wqb = _b(np.asarray(wq_w).T.reshape(DT, P, FT, P).transpose(2, 1, 0, 3))
    wkb = _b(np.asarray(wk_w).T.reshape(DT, P, FT, P).transpose(2, 1, 0, 3))
    wvb = _b(np.asarray(wv_w).T.reshape(DT, P, 2, 512).transpose(2, 1, 0, 3))
    bq_pb = _c(np.asarray(wq_b).reshape(FT, P).T)   # [P, FT]
    bk_pb = _c(np.asarray(wk_b).reshape(FT, P).T)
    bvB = _c(np.broadcast_to(np.asarray(wv_b), (P, F)))
    x_t = x.reshape(MT, P, D)                       # [tile, row, d]
    in_a = []
    for c in range(C):
        rows = x_t[c::C].reshape(NL, D)             # tiles c, 8+c, 16+c, 24+c
        xT_blk = _b(rows.T.reshape(DT, P, NL).transpose(1, 0, 2))
        in_a.append(
            {
                "xT": xT_blk,
                "wqb": wqb,
                "wkb": wkb,
                "wvb": wvb,
                "bq": bq_pb,
                "bk": bk_pb,
                "bvB": bvB,
            }
        )
    res_a = run_bass_kernel_spmd(nc_qkv, in_a, core_ids=list(range(C)))
    LAST_EXEC_NS[0] = res_a.exec_time_ns
    LAST_RESULTS[0] = res_a

    # reassemble full k/v in natural row order (tile index = 8*s + c)
    kT_all = np.stack(
        [np.asarray(res_a.results[c]["kT_o"]).reshape(F, SLOTS, P) for c in range(C)]
    )  # [c, F, s, P]
    kT_full = kT_all.transpose(1, 2, 0, 3).reshape(F, N)
    v_all = np.stack(
        [np.asarray(res_a.results[c]["v_o"]).reshape(SLOTS, P, F) for c in range(C)]
    )  # [c, s, P, F]
    v_full = v_all.transpose(1, 0, 2, 3).reshape(N, F)

    # ---- launch B: attention + projection (chunk-blocked, replicated k/v)
    # kb[ci, p, u, ft, j] = kT_full[ft*128 + p, (8*ci+u)*128 + j]
    kb = np.ascontiguousarray(
        kT_full.reshape(FT, P, CI, 8, P).transpose(2, 1, 3, 0, 4)
    )
    vb = np.ascontiguousarray(v_full.reshape(CI, 8, P, F).transpose(0, 2, 1, 3))
    projTb = _b(np.asarray(proj_w).T.reshape(FT, P, F).transpose(1, 0, 2))
    pbB = _c(np.broadcast_to(np.asarray(proj_b), (P, F)))
    tri = (np.arange(P)[:, None] <= np.arange(P)[None, :])  # key p <= query q
    in_b = []
    for c in range(C):
        qT_blk = np.ascontiguousarray(
            np.asarray(res_a.results[c]["qT_o"]).reshape(FT, P, NL).transpose(1, 0, 2)
        )
        maskb = np.zeros((P, 8, P), dtype=ml_dtypes.bfloat16)
        maskb[:, :c, :] = 1.0
        maskb[:, c, :] = tri.astype(ml_dtypes.bfloat16)
        in_b.append(
            {
                "qT": qT_blk,
                "kb": kb,
                "vb": vb,
                "maskb": maskb,
                "projTb": projTb,
                "pbB": pbB,
            }
        )
    res_b = run_bass_kernel_spmd(nc_attn, in_b, core_ids=list(range(C)))
    LAST_EXEC_NS[1] = res_b.exec_time_ns
    LAST_RESULTS[1] = res_b

    out_all = np.stack(
        [np.asarray(res_b.results[c]["out_o"]).reshape(SLOTS, P, F) for c in range(C)]
    )  # [c, k, P, F]
    return out_all.transpose(1, 0, 2, 3).reshape(N, F)
